# revision 9
# baseline (speedup 1.0000x reference)
"""Trainium2 Bass kernel for nn_Attention_89412629168340 (sparse_attention).

Reference computation (single-query attention over [T=4096, B=16, H=1024]):
    q   = inputs[lengths-1, b, :] @ Wq.T + bq            [B, H]
    k   = inputs @ Wk.T + bk                             [B, T, H]
    v   = inputs @ Wv.T + bv                             [B, T, H]
    s   = (q . k) / sqrt(H),  masked to t < lengths[b]   [B, T]
    w   = softmax(s)                                     [B, T]
    vw  = sum_t w * v                                    [B, H]
    out = vw @ We.T + be                                 [1, B, H]
    returns (out, w)

Algebraic restructure (removes the two T*H*H projections entirely):
    s[b,t]  = (inputs[t,b,:] . qk[b,:] + q[b].bk) / sqrt(H),  qk[b] = Wk.T @ q[b]
    vw[b]   = Wv @ (sum_t w[b,t] * inputs[t,b,:]) + bv        (since sum_t w = 1)
This turns the kernel memory-bound: each core streams its shard of `inputs`
exactly once (one-pass, unnormalized-exp softmax; scores are small so no
max-subtraction is needed).

Sharding: data-parallel over batch for the main loop (2 batches/core);
tensor-parallel (output-column slices) for the tiny q-chain and epilogue
matmuls so each core reads only 1/8 of each HxH weight. Three small
collectives: ReduceScatter(qk,c), AllGather(ctx), AllReduce(emb).
"""

import os
import sys

import numpy as np

for _p in ("/opt/trn_rl_repo", "/root/.axon_site/_ro/trn_rl_repo"):
    if os.path.isdir(_p) and _p not in sys.path:
        sys.path.insert(0, _p)

import concourse.bass as bass
import concourse.tile as tile
from concourse import bacc, mybir
from concourse.bass_utils import run_bass_kernel_spmd
from concourse.masks import make_identity

H = 1024
T = 4096
B = 16
NCORES = 8
BPC = B // NCORES          # batches per core = 2
NCOL = T // 128            # 32 P-tile columns per batch
NCH = T // 256             # 16 DMA chunks (256 t each) per batch
INV_NORM = 1.0 / 32.0      # 1/sqrt(H)
MASK_NEG = -30000.0   # exp underflows to exactly 0; safer for the ACT LUT than -1e10
f32 = mybir.dt.float32

_CACHE = {}


def _dap(handle, offset, dims):
    """DRAM access pattern: dims = [[step, count], ...] in elements."""
    base = handle[:]
    return bass.AP(tensor=base.tensor, offset=base.offset + offset, ap=dims)


def _bcast(dram_tile, offset_elems, nparts, count):
    """Broadcast `count` contiguous elements of a DRAM tile across nparts."""
    base = dram_tile[:]
    return bass.AP(
        tensor=base.tensor,
        offset=base.offset + offset_elems,
        ap=[[0, nparts], [1, count]],
    )


def build_nc(reps=1):
    nc = bacc.Bacc("TRN2", target_bir_lowering=False, debug=False,
                   num_devices=NCORES)

    # ---- I/O ----
    x = nc.declare_dram_parameter("x", [T, BPC, H], f32, isOutput=False)
    qinT = nc.declare_dram_parameter("qinT", [H, B], f32, isOutput=False)
    wqt = nc.declare_dram_parameter("wqt", [H, 128], f32, isOutput=False)
    wk = nc.declare_dram_parameter("wk", [128, H], f32, isOutput=False)
    wvt = nc.declare_dram_parameter("wvt", [H, 128], f32, isOutput=False)
    wet = nc.declare_dram_parameter("wet", [128, H], f32, isOutput=False)
    bq_sl = nc.declare_dram_parameter("bq_sl", [128, 1], f32, isOutput=False)
    bk_sl = nc.declare_dram_parameter("bk_sl", [128, 1], f32, isOutput=False)
    bv_sl = nc.declare_dram_parameter("bv_sl", [128, 1], f32, isOutput=False)
    be_full = nc.declare_dram_parameter("be_full", [1, H], f32, isOutput=False)
    maskb = nc.declare_dram_parameter("maskb", [BPC, 128, NCOL], f32,
                                      isOutput=False)
    w_out = nc.declare_dram_parameter("w_out", [BPC, T], f32, isOutput=True)
    emb_out = nc.declare_dram_parameter("emb_out", [B, H], f32, isOutput=True)

    grp = [list(range(NCORES))]

    with tile.TileContext(nc) as tc:
        with (
            tc.tile_pool(name="singles", bufs=1) as sg,
            tc.tile_pool(name="wpool", bufs=4) as wp,
            tc.tile_pool(name="xpool", bufs=4) as xp,
            tc.tile_pool(name="scpool", bufs=4) as scp,
            tc.tile_pool(name="jpool", bufs=1) as jp,
            tc.tile_pool(name="pbig", bufs=2, space="PSUM") as pbig,
            tc.tile_pool(name="psm", bufs=4, space="PSUM") as psm,
            tc.tile_pool(name="dram", bufs=1, space="DRAM") as dr,
        ):
            # ---- constants (hoisted out of the rep loop) ----
            ident = sg.tile([128, 128], f32)
            make_identity(nc, ident)
            ones_mat = sg.tile([128, 128], f32)
            nc.vector.memset(ones_mat, 1.0)

            # ---- small-input DMAs (loaded once) ----
            qinT_sb = sg.tile([128, 8, B], f32)
            nc.sync.dma_start(out=qinT_sb[:],
                              in_=_dap(qinT, 0, [[B, 128], [128 * B, 8], [1, B]]))
            wqt_sb = wp.tile([128, 8, 128], f32, tag="wqt")
            nc.sync.dma_start(out=wqt_sb[:],
                              in_=_dap(wqt, 0, [[128, 128], [128 * 128, 8], [1, 128]]))
            wk_sb = wp.tile([128, H], f32, tag="wk")
            nc.sync.dma_start(out=wk_sb[:], in_=wk[:])
            wvt_sb = wp.tile([128, 8, 128], f32, tag="wvt")
            nc.sync.dma_start(out=wvt_sb[:],
                              in_=_dap(wvt, 0, [[128, 128], [128 * 128, 8], [1, 128]]))
            wet_sb = wp.tile([128, H], f32, tag="wet")
            nc.sync.dma_start(out=wet_sb[:], in_=wet[:])
            bq_sb = sg.tile([128, 1], f32)
            nc.sync.dma_start(out=bq_sb[:], in_=bq_sl[:])
            bk_sb = sg.tile([128, 1], f32)
            nc.sync.dma_start(out=bk_sb[:], in_=bk_sl[:])
            bv_sb = sg.tile([128, 1], f32)
            nc.sync.dma_start(out=bv_sb[:], in_=bv_sl[:])
            be16 = sg.tile([16, H], f32)
            nc.gpsimd.dma_start(be16[:], _dap(be_full, 0, [[0, 16], [1, H]]))
            be8_sb = sg.tile([16, H], f32)
            nc.scalar.mul(be8_sb, be16, 1.0 / NCORES)
            mb_sb = []
            for b in range(BPC):
                t_mb = sg.tile([128, NCOL], f32, tag=f"mb{b}", name=f"mb{b}")
                nc.sync.dma_start(
                    out=t_mb[:],
                    in_=_dap(maskb, b * 128 * NCOL, [[NCOL, 128], [1, NCOL]]))
                mb_sb.append(t_mb)

            bk_scaled = sg.tile([128, 1], f32)
            nc.scalar.mul(bk_scaled, bk_sb, INV_NORM)

            for _rep in range(reps):
                # ---- prologue: q-chain (tensor-parallel over o-slice) ----
                # qT_slice[o, b] = sum_h WqT[h, o] * qinT[h, b] + bq[o]
                qT_ps = psm.tile([128, 16], f32, tag="sm", name="qT_ps")
                for hc in range(8):
                    nc.tensor.matmul(qT_ps, lhsT=wqt_sb[:, hc, :],
                                     rhs=qinT_sb[:, hc, :],
                                     start=(hc == 0), stop=(hc == 7))
                qT_sb = sg.tile([128, 16], f32, tag="qT_sb", name="qT_sb")
                nc.vector.tensor_scalar_add(qT_sb, qT_ps, bq_sb)

                # qk partial: [16, H] = sum_{o in slice} q[b, o] * Wk[o, h]
                # c partial:  [16, 1] = sum_{o in slice} q[b, o] * bk[o] / 32
                qk0 = psm.tile([16, 512], f32, tag="sm", name="qk0")
                qk1 = psm.tile([16, 512], f32, tag="sm", name="qk1")
                c_ps = psm.tile([16, 8], f32, tag="sm", name="c_ps")
                nc.tensor.matmul(qk0, lhsT=qT_sb, rhs=wk_sb[:, 0:512],
                                 start=True, stop=True)
                nc.tensor.matmul(qk1, lhsT=qT_sb, rhs=wk_sb[:, 512:1024],
                                 start=True, stop=True)
                nc.tensor.matmul(c_ps[:, 0:1], lhsT=qT_sb, rhs=bk_scaled,
                                 start=True, stop=True)
                qkc_sb = sg.tile([16, 1032], f32, tag="qkc_sb", name="qkc_sb")
                nc.vector.memset(qkc_sb[:, 1024:1032], 0.0)
                nc.scalar.copy(qkc_sb[:, 0:512], qk0)
                nc.scalar.copy(qkc_sb[:, 512:1024], qk1)
                nc.scalar.copy(qkc_sb[:, 1024:1025], c_ps[:, 0:1])

                qkc_part = dr.tile([16, 1032], f32, tag="qkc_part",
                                   name="qkc_part")
                qkc_rs = dr.tile([BPC, 1032], f32, tag="qkc_rs", name="qkc_rs")
                nc.gpsimd.dma_start(qkc_part[:], qkc_sb[:])
                nc.gpsimd.collective_compute(
                    "ReduceScatter", mybir.AluOpType.add, replica_groups=grp,
                    ins=[qkc_part.opt()], outs=[qkc_rs.opt()])

                # per-batch replicated qk row and c scalar
                qkb = []
                ebias = []
                for b in range(BPC):
                    t_qk = sg.tile([128, H], f32, tag=f"qkb{b}", name=f"qkb{b}")
                    nc.gpsimd.dma_start(t_qk[:], _bcast(qkc_rs, b * 1032, 128, H))
                    qkb.append(t_qk)
                    t_c = sg.tile([128, 1], f32, tag=f"cb{b}", name=f"cb{b}")
                    nc.gpsimd.dma_start(t_c[:],
                                        _bcast(qkc_rs, b * 1032 + 1024, 128, 1))
                    t_eb = sg.tile([128, NCOL], f32, tag=f"eb{b}", name=f"eb{b}")
                    nc.vector.tensor_scalar_add(t_eb, mb_sb[b], t_c)
                    ebias.append(t_eb)

                # ---- main loop: one pass over x ----
                P_t = [sg.tile([128, NCOL], f32, tag=f"P{b}", name=f"P{b}")
                       for b in range(BPC)]
                ctx_part = dr.tile([BPC, H], f32, tag="ctx_part", name="ctx_part")
                for b in range(BPC):
                    acc = pbig.tile([1, H], f32, tag="big", name="acc")
                    for tch in range(NCH):
                        xt = xp.tile([128, 2, H], f32, tag="x", name="xt")
                        t0 = tch * 256
                        nc.sync.dma_start(
                            out=xt[:],
                            in_=_dap(x, t0 * BPC * H + b * H,
                                     [[BPC * H, 128], [128 * BPC * H, 2], [1, H]]))
                        for s in range(2):
                            col = tch * 2 + s
                            jnk = jp.tile([128, H], f32, tag="jnk", name="jnk")
                            sc = scp.tile([128, 1], f32, tag="sc", name="sc")
                            nc.vector.tensor_mul(jnk, xt[:, s, :], qkb[b])
                            jnk2 = jp.tile([128, H], f32, tag="jnk2",
                                           name="jnk2")
                            nc.scalar.activation(
                                jnk2, jnk, mybir.ActivationFunctionType.Copy,
                                bias=0.0, scale=INV_NORM, accum_out=sc)
                            nc.scalar.activation(
                                P_t[b][:, col:col + 1], sc,
                                mybir.ActivationFunctionType.Exp,
                                bias=ebias[b][:, col:col + 1], scale=1.0)
                            nc.tensor.matmul(acc[:, 0:512],
                                             lhsT=P_t[b][:, col:col + 1],
                                             rhs=xt[:, s, 0:512],
                                             start=(col == 0),
                                             stop=(col == NCOL - 1))
                            nc.tensor.matmul(acc[:, 512:1024],
                                             lhsT=P_t[b][:, col:col + 1],
                                             rhs=xt[:, s, 512:1024],
                                             start=(col == 0),
                                             stop=(col == NCOL - 1))

                    # ---- per-batch epilogue: denom, ctx, attention weights ----
                    colsum = scp.tile([128, 1], f32, tag="colsum", name="colsum")
                    nc.vector.reduce_sum(out=colsum, in_=P_t[b],
                                         axis=mybir.AxisListType.X)
                    d_ps = psm.tile([128, 8], f32, tag="sm", name="d_ps")
                    nc.tensor.matmul(d_ps[:, 0:1], lhsT=ones_mat, rhs=colsum,
                                     start=True, stop=True)
                    r128_sb = sg.tile([128, 1], f32, tag=f"r{b}", name=f"r{b}")
                    nc.vector.reciprocal(r128_sb, d_ps[:, 0:1])
                    ctx_sb = sg.tile([1, H], f32, tag=f"ctx{b}", name=f"ctx{b}")
                    nc.scalar.activation(ctx_sb, acc,
                                         mybir.ActivationFunctionType.Copy,
                                         bias=0.0, scale=r128_sb[0:1, :])
                    nc.gpsimd.dma_start(
                        _dap(ctx_part, b * H, [[H, 1], [1, H]]), ctx_sb[:])

                    # w = P / denom, written [32, 128] row-major in t
                    w_ps = psm.tile([32, 128], f32, tag="sm", name="w_ps")
                    nc.tensor.transpose(w_ps, P_t[b], ident)
                    w_sb = sg.tile([32, 128], f32, tag=f"w{b}", name=f"w{b}")
                    nc.scalar.activation(w_sb, w_ps,
                                         mybir.ActivationFunctionType.Copy,
                                         bias=0.0, scale=r128_sb[0:32, :])
                    nc.sync.dma_start(
                        out=_dap(w_out, b * T, [[128, 32], [1, 128]]),
                        in_=w_sb[:])

                # ---- global epilogue ----
                ctx_full = dr.tile([B, H], f32, tag="ctx_full", name="ctx_full")
                nc.gpsimd.collective_compute(
                    "AllGather", mybir.AluOpType.bypass, replica_groups=grp,
                    ins=[ctx_part.opt()], outs=[ctx_full.opt()])
                ctx32_sb = sg.tile([32, H], f32, tag="ctx32", name="ctx32")
                nc.vector.memset(ctx32_sb[:], 0.0)
                nc.sync.dma_start(out=ctx32_sb[0:16, :], in_=ctx_full[:])
                ctxT_sb = sg.tile([128, 8, 16], f32, tag="ctxT", name="ctxT")
                for hc in range(8):
                    ct_ps = psm.tile([128, 32], f32, tag="sm", name="ct_ps")
                    nc.tensor.transpose(ct_ps,
                                        ctx32_sb[:, hc * 128:(hc + 1) * 128],
                                        ident[0:32, 0:32])
                    nc.scalar.copy(ctxT_sb[:, hc, :], ct_ps[:, 0:16])

                # vw slice: [o128, 16] = bv[o] + sum_h WvT[h, o] * ctxT[h, b]
                vw_ps = psm.tile([128, 16], f32, tag="sm", name="vw_ps")
                for hc in range(8):
                    nc.tensor.matmul(vw_ps, lhsT=wvt_sb[:, hc, :],
                                     rhs=ctxT_sb[:, hc, :],
                                     start=(hc == 0), stop=(hc == 7))
                vw_sb = sg.tile([128, 16], f32, tag="vw_sb", name="vw_sb")
                nc.vector.tensor_scalar_add(vw_sb, vw_ps, bv_sb)

                # emb partial: [16, H] = be/8 + sum_{o in slice} vw[o,b]*WeT[o,h]
                emb_ps = pbig.tile([16, H], f32, tag="big", name="emb_ps")
                nc.tensor.matmul(emb_ps[:, 0:512], lhsT=vw_sb,
                                 rhs=wet_sb[:, 0:512], start=True, stop=True)
                nc.tensor.matmul(emb_ps[:, 512:1024], lhsT=vw_sb,
                                 rhs=wet_sb[:, 512:1024], start=True, stop=True)
                emb_sb = sg.tile([16, H], f32, tag="emb_sb", name="emb_sb")
                nc.vector.tensor_add(emb_sb, emb_ps, be8_sb)

                emb_part = dr.tile([B, H], f32, tag="emb_part", name="emb_part")
                emb_full = dr.tile([B, H], f32, tag="emb_full", name="emb_full")
                nc.gpsimd.dma_start(emb_part[:], emb_sb[:])
                nc.gpsimd.collective_compute(
                    "AllReduce", mybir.AluOpType.add, replica_groups=grp,
                    ins=[emb_part.opt()], outs=[emb_full.opt()])
                nc.gpsimd.dma_start(emb_out[:], emb_full[:])

    nc.finalize()
    return nc


def make_in_maps(inputs, lengths, Wq, bq, Wk, bk, Wv, bv, We, be):
    inputs = np.ascontiguousarray(np.asarray(inputs, dtype=np.float32))
    Wq = np.asarray(Wq, dtype=np.float32)
    bq = np.asarray(bq, dtype=np.float32)
    Wk = np.asarray(Wk, dtype=np.float32)
    bk = np.asarray(bk, dtype=np.float32)
    Wv = np.asarray(Wv, dtype=np.float32)
    bv = np.asarray(bv, dtype=np.float32)
    We = np.asarray(We, dtype=np.float32)
    be = np.asarray(be, dtype=np.float32)

    t, b, h = inputs.shape
    assert (t, b, h) == (T, B, H)

    idx = np.asarray(lengths).astype(np.int64) - 1
    q_in = inputs[idx, np.arange(B), :]                     # [B, H]
    qinT = np.ascontiguousarray(q_in.T)                     # [H, B]

    tt = np.arange(T)
    valid = tt[None, :] < np.asarray(lengths).astype(np.int64)[:, None]
    maskbias = np.where(valid, 0.0, MASK_NEG).astype(np.float32)  # [B, T]

    in_maps = []
    for i in range(NCORES):
        osl = slice(128 * i, 128 * (i + 1))
        bsl = slice(BPC * i, BPC * (i + 1))
        mb = maskbias[bsl].reshape(BPC, NCOL, 128).transpose(0, 2, 1)
        in_maps.append({
            "x": np.ascontiguousarray(inputs[:, bsl, :]),
            "qinT": qinT,
            "wqt": np.ascontiguousarray(Wq[osl, :].T),
            "wk": np.ascontiguousarray(Wk[osl, :]),
            "wvt": np.ascontiguousarray(Wv[osl, :].T),
            "wet": np.ascontiguousarray(We[:, osl].T),
            "bq_sl": bq[osl].reshape(128, 1),
            "bk_sl": bk[osl].reshape(128, 1),
            "bv_sl": bv[osl].reshape(128, 1),
            "be_full": be.reshape(1, H),
            "maskb": np.ascontiguousarray(mb),
        })
    return in_maps


def kernel(inputs, lengths, Wq, bq, Wk, bk, Wv, bv, We, be):
    in_maps = make_in_maps(inputs, lengths, Wq, bq, Wk, bk, Wv, bv, We, be)
    if "nc" not in _CACHE:
        _CACHE["nc"] = build_nc()
    nc = _CACHE["nc"]
    res = run_bass_kernel_spmd(nc, in_maps, list(range(NCORES)))

    attention_weights = np.concatenate(
        [res.results[i]["w_out"] for i in range(NCORES)], axis=0)  # [B, T]
    output = res.results[0]["emb_out"][None, :, :]                 # [1, B, H]
    return output, attention_weights


# revision 10
# speedup vs baseline: 3.4762x; 3.4762x over previous
"""Trainium2 Bass kernel for nn_Attention_89412629168340 (sparse_attention).

Reference computation (single-query attention over [T=4096, B=16, H=1024]):
    q   = inputs[lengths-1, b, :] @ Wq.T + bq            [B, H]
    k   = inputs @ Wk.T + bk                             [B, T, H]
    v   = inputs @ Wv.T + bv                             [B, T, H]
    s   = (q . k) / sqrt(H),  masked to t < lengths[b]   [B, T]
    w   = softmax(s)                                     [B, T]
    vw  = sum_t w * v                                    [B, H]
    out = vw @ We.T + be                                 [1, B, H]
    returns (out, w)

Algebraic restructure (removes the two T*H*H projections entirely):
    s[b,t]  = (inputs[t,b,:] . qk[b,:] + q[b].bk) / sqrt(H),  qk[b] = Wk.T @ q[b]
    vw[b]   = Wv @ (sum_t w[b,t] * inputs[t,b,:]) + bv        (since sum_t w = 1)
This turns the kernel memory-bound: each core streams its shard of `inputs`
exactly once (one-pass, unnormalized-exp softmax; scores are small so no
max-subtraction is needed).

Sharding: data-parallel over batch for the main loop (2 batches/core);
tensor-parallel (output-column slices) for the tiny q-chain and epilogue
matmuls so each core reads only 1/8 of each HxH weight. Three small
collectives: ReduceScatter(qk,c), AllGather(ctx), AllReduce(emb).
"""

import os
import sys

import numpy as np

for _p in ("/opt/trn_rl_repo", "/root/.axon_site/_ro/trn_rl_repo"):
    if os.path.isdir(_p) and _p not in sys.path:
        sys.path.insert(0, _p)

import concourse.bass as bass
import concourse.tile as tile
from concourse import bacc, mybir
from concourse.bass_utils import run_bass_kernel_spmd
from concourse.masks import make_identity

H = 1024
T = 4096
B = 16
NCORES = 8
BPC = B // NCORES          # batches per core = 2
NCOL = T // 128            # 32 P-tile columns per batch
NCH = T // 256             # 16 DMA chunks (256 t each) per batch
INV_NORM = 1.0 / 32.0      # 1/sqrt(H)
MASK_NEG = -30000.0   # exp underflows to exactly 0; safer for the ACT LUT than -1e10
f32 = mybir.dt.float32

_CACHE = {}


def _dap(handle, offset, dims):
    """DRAM access pattern: dims = [[step, count], ...] in elements."""
    base = handle[:]
    return bass.AP(tensor=base.tensor, offset=base.offset + offset, ap=dims)


def _bcast(dram_tile, offset_elems, nparts, count):
    """Broadcast `count` contiguous elements of a DRAM tile across nparts."""
    base = dram_tile[:]
    return bass.AP(
        tensor=base.tensor,
        offset=base.offset + offset_elems,
        ap=[[0, nparts], [1, count]],
    )


def build_nc(reps=1):
    nc = bacc.Bacc("TRN2", target_bir_lowering=False, debug=False,
                   num_devices=NCORES)

    # ---- I/O ----
    x = nc.declare_dram_parameter("x", [T, BPC, H], f32, isOutput=False)
    qinT = nc.declare_dram_parameter("qinT", [H, B], f32, isOutput=False)
    wqt = nc.declare_dram_parameter("wqt", [H, 128], f32, isOutput=False)
    wk = nc.declare_dram_parameter("wk", [128, H], f32, isOutput=False)
    wvt = nc.declare_dram_parameter("wvt", [H, 128], f32, isOutput=False)
    wet = nc.declare_dram_parameter("wet", [128, H], f32, isOutput=False)
    bq_sl = nc.declare_dram_parameter("bq_sl", [128, 1], f32, isOutput=False)
    bk_sl = nc.declare_dram_parameter("bk_sl", [128, 1], f32, isOutput=False)
    bv_sl = nc.declare_dram_parameter("bv_sl", [128, 1], f32, isOutput=False)
    be_full = nc.declare_dram_parameter("be_full", [1, H], f32, isOutput=False)
    maskb = nc.declare_dram_parameter("maskb", [BPC, 128, NCOL], f32,
                                      isOutput=False)
    w_out = nc.declare_dram_parameter("w_out", [BPC, T], f32, isOutput=True)
    emb_out = nc.declare_dram_parameter("emb_out", [B, H], f32, isOutput=True)

    grp = [list(range(NCORES))]

    with tile.TileContext(nc) as tc:
        with (
            tc.tile_pool(name="singles", bufs=1) as sg,
            tc.tile_pool(name="wpool", bufs=4) as wp,
            tc.tile_pool(name="xpool", bufs=6) as xp,
            tc.tile_pool(name="scpool", bufs=4) as scp,
            tc.tile_pool(name="jpool", bufs=2) as jp,
            tc.tile_pool(name="pbig", bufs=2, space="PSUM") as pbig,
            tc.tile_pool(name="psm", bufs=4, space="PSUM") as psm,
            tc.tile_pool(name="dram", bufs=1, space="DRAM") as dr,
        ):
            # ---- constants (hoisted out of the rep loop) ----
            ident = sg.tile([128, 128], f32)
            make_identity(nc, ident)
            ones_mat = sg.tile([128, 128], f32)
            nc.vector.memset(ones_mat, 1.0)

            # ---- small-input DMAs (loaded once) ----
            qinT_sb = sg.tile([128, 8, B], f32)
            nc.sync.dma_start(out=qinT_sb[:],
                              in_=_dap(qinT, 0, [[B, 128], [128 * B, 8], [1, B]]))
            wqt_sb = wp.tile([128, 8, 128], f32, tag="wqt")
            nc.sync.dma_start(out=wqt_sb[:],
                              in_=_dap(wqt, 0, [[128, 128], [128 * 128, 8], [1, 128]]))
            wk_sb = wp.tile([128, H], f32, tag="wk")
            nc.sync.dma_start(out=wk_sb[:], in_=wk[:])
            wvt_sb = wp.tile([128, 8, 128], f32, tag="wvt")
            nc.sync.dma_start(out=wvt_sb[:],
                              in_=_dap(wvt, 0, [[128, 128], [128 * 128, 8], [1, 128]]))
            wet_sb = wp.tile([128, H], f32, tag="wet")
            nc.sync.dma_start(out=wet_sb[:], in_=wet[:])
            bq_sb = sg.tile([128, 1], f32)
            nc.sync.dma_start(out=bq_sb[:], in_=bq_sl[:])
            bk_sb = sg.tile([128, 1], f32)
            nc.sync.dma_start(out=bk_sb[:], in_=bk_sl[:])
            bv_sb = sg.tile([128, 1], f32)
            nc.sync.dma_start(out=bv_sb[:], in_=bv_sl[:])
            be16 = sg.tile([16, H], f32)
            nc.gpsimd.dma_start(be16[:], _dap(be_full, 0, [[0, 16], [1, H]]))
            be8_sb = sg.tile([16, H], f32)
            nc.scalar.mul(be8_sb, be16, 1.0 / NCORES)
            mb_sb = []
            for b in range(BPC):
                t_mb = sg.tile([128, NCOL], f32, tag=f"mb{b}", name=f"mb{b}")
                nc.sync.dma_start(
                    out=t_mb[:],
                    in_=_dap(maskb, b * 128 * NCOL, [[NCOL, 128], [1, NCOL]]))
                mb_sb.append(t_mb)

            bk_scaled = sg.tile([128, 1], f32)
            nc.scalar.mul(bk_scaled, bk_sb, INV_NORM)

            for _rep in range(reps):
                # ---- prologue: q-chain (tensor-parallel over o-slice) ----
                # qT_slice[o, b] = sum_h WqT[h, o] * qinT[h, b] + bq[o]
                qT_ps = psm.tile([128, 16], f32, tag="sm", name="qT_ps")
                for hc in range(8):
                    nc.tensor.matmul(qT_ps, lhsT=wqt_sb[:, hc, :],
                                     rhs=qinT_sb[:, hc, :],
                                     start=(hc == 0), stop=(hc == 7))
                qT_sb = sg.tile([128, 16], f32, tag="qT_sb", name="qT_sb")
                nc.vector.tensor_scalar_add(qT_sb, qT_ps, bq_sb)

                # qk partial: [16, H] = sum_{o in slice} q[b, o] * Wk[o, h]
                # c partial:  [16, 1] = sum_{o in slice} q[b, o] * bk[o] / 32
                qk0 = psm.tile([16, 512], f32, tag="sm", name="qk0")
                qk1 = psm.tile([16, 512], f32, tag="sm", name="qk1")
                c_ps = psm.tile([16, 8], f32, tag="sm", name="c_ps")
                nc.tensor.matmul(qk0, lhsT=qT_sb, rhs=wk_sb[:, 0:512],
                                 start=True, stop=True)
                nc.tensor.matmul(qk1, lhsT=qT_sb, rhs=wk_sb[:, 512:1024],
                                 start=True, stop=True)
                nc.tensor.matmul(c_ps[:, 0:1], lhsT=qT_sb, rhs=bk_scaled,
                                 start=True, stop=True)
                qkc_sb = sg.tile([16, 1032], f32, tag="qkc_sb", name="qkc_sb")
                nc.vector.memset(qkc_sb[:, 1024:1032], 0.0)
                nc.scalar.copy(qkc_sb[:, 0:512], qk0)
                nc.scalar.copy(qkc_sb[:, 512:1024], qk1)
                nc.scalar.copy(qkc_sb[:, 1024:1025], c_ps[:, 0:1])

                qkc_part = dr.tile([16, 1032], f32, tag="qkc_part",
                                   name="qkc_part")
                qkc_rs = dr.tile([BPC, 1032], f32, tag="qkc_rs", name="qkc_rs")
                nc.gpsimd.dma_start(qkc_part[:], qkc_sb[:])
                nc.gpsimd.collective_compute(
                    "ReduceScatter", mybir.AluOpType.add, replica_groups=grp,
                    ins=[qkc_part.opt()], outs=[qkc_rs.opt()])

                # per-batch replicated qk row and c scalar
                qkb = []
                ebias = []
                for b in range(BPC):
                    t_qk = sg.tile([128, H], f32, tag=f"qkb{b}", name=f"qkb{b}")
                    nc.gpsimd.dma_start(t_qk[:], _bcast(qkc_rs, b * 1032, 128, H))
                    qkb.append(t_qk)
                    t_c = sg.tile([128, 1], f32, tag=f"cb{b}", name=f"cb{b}")
                    nc.gpsimd.dma_start(t_c[:],
                                        _bcast(qkc_rs, b * 1032 + 1024, 128, 1))
                    t_eb = sg.tile([128, NCOL], f32, tag=f"eb{b}", name=f"eb{b}")
                    nc.vector.tensor_scalar_add(t_eb, mb_sb[b], t_c)
                    ebias.append(t_eb)

                # ---- main loop: one pass over x ----
                P_t = [sg.tile([128, NCOL], f32, tag=f"P{b}", name=f"P{b}")
                       for b in range(BPC)]
                ctx_part = dr.tile([BPC, H], f32, tag="ctx_part", name="ctx_part")
                for b in range(BPC):
                    acc = pbig.tile([1, H], f32, tag="big", name="acc")
                    for tch in range(NCH):
                        xt = xp.tile([128, 2, H], f32, tag="x", name="xt")
                        t0 = tch * 256
                        nc.sync.dma_start(
                            out=xt[:],
                            in_=_dap(x, t0 * BPC * H + b * H,
                                     [[BPC * H, 128], [128 * BPC * H, 2], [1, H]]))
                        for s in range(2):
                            col = tch * 2 + s
                            jnk = jp.tile([128, H], f32, tag="jnk", name="jnk")
                            sc = scp.tile([128, 1], f32, tag="sc", name="sc")
                            nc.vector.tensor_mul(jnk, xt[:, s, :], qkb[b])
                            jnk2 = jp.tile([128, H], f32, tag="jnk2",
                                           name="jnk2")
                            nc.scalar.activation(
                                jnk2, jnk, mybir.ActivationFunctionType.Copy,
                                bias=0.0, scale=INV_NORM, accum_out=sc)
                            nc.scalar.activation(
                                P_t[b][:, col:col + 1], sc,
                                mybir.ActivationFunctionType.Exp,
                                bias=ebias[b][:, col:col + 1], scale=1.0)
                            nc.tensor.matmul(acc[:, 0:512],
                                             lhsT=P_t[b][:, col:col + 1],
                                             rhs=xt[:, s, 0:512],
                                             start=(col == 0),
                                             stop=(col == NCOL - 1))
                            nc.tensor.matmul(acc[:, 512:1024],
                                             lhsT=P_t[b][:, col:col + 1],
                                             rhs=xt[:, s, 512:1024],
                                             start=(col == 0),
                                             stop=(col == NCOL - 1))

                    # ---- per-batch epilogue: denom, ctx, attention weights ----
                    colsum = scp.tile([128, 1], f32, tag="colsum", name="colsum")
                    nc.vector.reduce_sum(out=colsum, in_=P_t[b],
                                         axis=mybir.AxisListType.X)
                    d_ps = psm.tile([128, 8], f32, tag="sm", name="d_ps")
                    nc.tensor.matmul(d_ps[:, 0:1], lhsT=ones_mat, rhs=colsum,
                                     start=True, stop=True)
                    r128_sb = sg.tile([128, 1], f32, tag=f"r{b}", name=f"r{b}")
                    nc.vector.reciprocal(r128_sb, d_ps[:, 0:1])
                    ctx_sb = sg.tile([1, H], f32, tag=f"ctx{b}", name=f"ctx{b}")
                    nc.scalar.activation(ctx_sb, acc,
                                         mybir.ActivationFunctionType.Copy,
                                         bias=0.0, scale=r128_sb[0:1, :])
                    nc.gpsimd.dma_start(
                        _dap(ctx_part, b * H, [[H, 1], [1, H]]), ctx_sb[:])

                    # w = P / denom, written [32, 128] row-major in t
                    w_ps = psm.tile([32, 128], f32, tag="sm", name="w_ps")
                    nc.tensor.transpose(w_ps, P_t[b], ident)
                    w_sb = sg.tile([32, 128], f32, tag=f"w{b}", name=f"w{b}")
                    nc.scalar.activation(w_sb, w_ps,
                                         mybir.ActivationFunctionType.Copy,
                                         bias=0.0, scale=r128_sb[0:32, :])
                    nc.sync.dma_start(
                        out=_dap(w_out, b * T, [[128, 32], [1, 128]]),
                        in_=w_sb[:])

                # ---- global epilogue ----
                ctx_full = dr.tile([B, H], f32, tag="ctx_full", name="ctx_full")
                nc.gpsimd.collective_compute(
                    "AllGather", mybir.AluOpType.bypass, replica_groups=grp,
                    ins=[ctx_part.opt()], outs=[ctx_full.opt()])
                ctx32_sb = sg.tile([32, H], f32, tag="ctx32", name="ctx32")
                nc.vector.memset(ctx32_sb[:], 0.0)
                nc.sync.dma_start(out=ctx32_sb[0:16, :], in_=ctx_full[:])
                ctxT_sb = sg.tile([128, 8, 16], f32, tag="ctxT", name="ctxT")
                for hc in range(8):
                    ct_ps = psm.tile([128, 32], f32, tag="sm", name="ct_ps")
                    nc.tensor.transpose(ct_ps,
                                        ctx32_sb[:, hc * 128:(hc + 1) * 128],
                                        ident[0:32, 0:32])
                    nc.scalar.copy(ctxT_sb[:, hc, :], ct_ps[:, 0:16])

                # vw slice: [o128, 16] = bv[o] + sum_h WvT[h, o] * ctxT[h, b]
                vw_ps = psm.tile([128, 16], f32, tag="sm", name="vw_ps")
                for hc in range(8):
                    nc.tensor.matmul(vw_ps, lhsT=wvt_sb[:, hc, :],
                                     rhs=ctxT_sb[:, hc, :],
                                     start=(hc == 0), stop=(hc == 7))
                vw_sb = sg.tile([128, 16], f32, tag="vw_sb", name="vw_sb")
                nc.vector.tensor_scalar_add(vw_sb, vw_ps, bv_sb)

                # emb partial: [16, H] = be/8 + sum_{o in slice} vw[o,b]*WeT[o,h]
                emb_ps = pbig.tile([16, H], f32, tag="big", name="emb_ps")
                nc.tensor.matmul(emb_ps[:, 0:512], lhsT=vw_sb,
                                 rhs=wet_sb[:, 0:512], start=True, stop=True)
                nc.tensor.matmul(emb_ps[:, 512:1024], lhsT=vw_sb,
                                 rhs=wet_sb[:, 512:1024], start=True, stop=True)
                emb_sb = sg.tile([16, H], f32, tag="emb_sb", name="emb_sb")
                nc.vector.tensor_add(emb_sb, emb_ps, be8_sb)

                emb_part = dr.tile([B, H], f32, tag="emb_part", name="emb_part")
                emb_full = dr.tile([B, H], f32, tag="emb_full", name="emb_full")
                nc.gpsimd.dma_start(emb_part[:], emb_sb[:])
                nc.gpsimd.collective_compute(
                    "AllReduce", mybir.AluOpType.add, replica_groups=grp,
                    ins=[emb_part.opt()], outs=[emb_full.opt()])
                nc.gpsimd.dma_start(emb_out[:], emb_full[:])

    nc.finalize()
    return nc


def make_in_maps(inputs, lengths, Wq, bq, Wk, bk, Wv, bv, We, be):
    inputs = np.ascontiguousarray(np.asarray(inputs, dtype=np.float32))
    Wq = np.asarray(Wq, dtype=np.float32)
    bq = np.asarray(bq, dtype=np.float32)
    Wk = np.asarray(Wk, dtype=np.float32)
    bk = np.asarray(bk, dtype=np.float32)
    Wv = np.asarray(Wv, dtype=np.float32)
    bv = np.asarray(bv, dtype=np.float32)
    We = np.asarray(We, dtype=np.float32)
    be = np.asarray(be, dtype=np.float32)

    t, b, h = inputs.shape
    assert (t, b, h) == (T, B, H)

    idx = np.asarray(lengths).astype(np.int64) - 1
    q_in = inputs[idx, np.arange(B), :]                     # [B, H]
    qinT = np.ascontiguousarray(q_in.T)                     # [H, B]

    tt = np.arange(T)
    valid = tt[None, :] < np.asarray(lengths).astype(np.int64)[:, None]
    maskbias = np.where(valid, 0.0, MASK_NEG).astype(np.float32)  # [B, T]

    in_maps = []
    for i in range(NCORES):
        osl = slice(128 * i, 128 * (i + 1))
        bsl = slice(BPC * i, BPC * (i + 1))
        mb = maskbias[bsl].reshape(BPC, NCOL, 128).transpose(0, 2, 1)
        in_maps.append({
            "x": np.ascontiguousarray(inputs[:, bsl, :]),
            "qinT": qinT,
            "wqt": np.ascontiguousarray(Wq[osl, :].T),
            "wk": np.ascontiguousarray(Wk[osl, :]),
            "wvt": np.ascontiguousarray(Wv[osl, :].T),
            "wet": np.ascontiguousarray(We[:, osl].T),
            "bq_sl": bq[osl].reshape(128, 1),
            "bk_sl": bk[osl].reshape(128, 1),
            "bv_sl": bv[osl].reshape(128, 1),
            "be_full": be.reshape(1, H),
            "maskb": np.ascontiguousarray(mb),
        })
    return in_maps


def kernel(inputs, lengths, Wq, bq, Wk, bk, Wv, bv, We, be):
    in_maps = make_in_maps(inputs, lengths, Wq, bq, Wk, bk, Wv, bv, We, be)
    if "nc" not in _CACHE:
        _CACHE["nc"] = build_nc()
    nc = _CACHE["nc"]
    res = run_bass_kernel_spmd(nc, in_maps, list(range(NCORES)))

    attention_weights = np.concatenate(
        [res.results[i]["w_out"] for i in range(NCORES)], axis=0)  # [B, T]
    output = res.results[0]["emb_out"][None, :, :]                 # [1, B, H]
    return output, attention_weights
